# revision 19
# baseline (speedup 1.0000x reference)
"""Multi-head attention (B=4,S=2048,D=1024,H=16,Hd=64, fp32) on 8 TRN2 NeuronCores.

Sharding: core c handles batch b=c//2 and query-row half h=c%2 (1024 rows).
Each core computes K/V for its full batch (2048 keys), Q for its 1024 rows,
full 16-head attention for those rows, and the output projection. No
collectives; the host gathers per-core [1024,1024] output^T slices. The host
rotates each core's x^T so the core's own rows sit in columns 0-1023 (key
order is irrelevant: attention reduces over keys), keeping the program SPMD.

v2 notes (trace-driven): x and Wq/Wk/Wv stream as bf16 (same 1 cycle/row PE
rate as f32r, half the DMA + SBUF). Back-to-back matmuls of one mode run at
theory speed but every fp32r/bf16/row_grp mode switch costs ~100ns, so the
slot interleave is chunked (scores 4 / proj 8 / ctx 8 instructions per turn)
instead of per-instruction. Q/K projections use 512-row moving x tiles.
Softmax normalization is batched per head: stage sums row, GpSimd
partition-broadcast [64,1024], reciprocal at 64-partition parallelism, one
multiply - pipelined via bufs=2 pools so the PE never waits on the old
serialized copy/recip/broadcast/mul chains. Out-proj interleaves with the
final ctx drain. Weight/x DMAs split in halves, wpool bufs=4 to cover the
projection-phase prefetch underruns.
"""
import numpy as np
import ml_dtypes
from contextlib import ExitStack

import concourse.bass as bass
import concourse.tile as tile
from concourse import bacc, mybir
from concourse.bass import ts, ds
from concourse.bass_utils import run_bass_kernel_spmd

P = 128
D = 1024
KC = 8                 # contraction chunks of 128
S = 2048               # keys per batch
R = 1024               # query rows per core
NB = 4                 # key blocks
SBK = S // NB          # 512 keys per block
KTB = SBK // P         # 4 key tiles per block
H = 16
HP = H // 2            # 8 head pairs
HD = 64
MP = 4                 # weight DMA tiles of 256 output-cols
BF16 = mybir.dt.bfloat16
F32 = mybir.dt.float32
FP = mybir.ActivationFunctionType

_CACHED = {}


def build():
    if "nc" in _CACHED:
        return _CACHED["nc"]
    nc = bacc.Bacc("TRN2", target_bir_lowering=False, debug=False, num_devices=8)
    xTt = nc.dram_tensor("xTt", [P, 4, KC, 512], BF16, kind="ExternalInput").ap()
    xTtK = nc.dram_tensor("xTtK", [P, 4, KC, 512], BF16, kind="ExternalInput").ap()
    Wq4 = nc.dram_tensor("Wq4", [P, MP, KC, 256], BF16, kind="ExternalInput").ap()
    Wk4 = nc.dram_tensor("Wk4", [P, MP, KC, 256], BF16, kind="ExternalInput").ap()
    Wv4 = nc.dram_tensor("Wv4", [P, MP, KC, 256], BF16, kind="ExternalInput").ap()
    Wo4 = nc.dram_tensor("Wo4", [P, MP, KC, 256], BF16, kind="ExternalInput").ap()
    Wk4h = nc.dram_tensor("Wk4h", [P, 2, KC, 256], BF16, kind="ExternalInput").ap()
    Wv4h = nc.dram_tensor("Wv4h", [P, 2, KC, 256], BF16, kind="ExternalInput").ap()
    bkh = nc.dram_tensor("bkh", [P, 4], F32, kind="ExternalInput").ap()
    bvbh = nc.dram_tensor("bvbh", [P, 512], F32, kind="ExternalInput").ap()
    bq = nc.dram_tensor("bq", [P, KC], F32, kind="ExternalInput").ap()
    bk = nc.dram_tensor("bk", [P, KC], F32, kind="ExternalInput").ap()
    bo = nc.dram_tensor("bo", [P, KC], F32, kind="ExternalInput").ap()
    bvb = nc.dram_tensor("bvb", [P, D], F32, kind="ExternalInput").ap()
    outT = nc.dram_tensor("outT", [D, R], BF16, kind="ExternalOutput").ap()

    def wdma(wt, src, mp):
        nc.sync.dma_start(wt[:, 0:4], src[:, mp, 0:4])
        nc.sync.dma_start(wt[:, 4:8], src[:, mp, 4:8])

    with tile.TileContext(nc) as tc:
        with ExitStack() as ctx:
            const = ctx.enter_context(tc.tile_pool(name="const", bufs=1))
            wpool = ctx.enter_context(tc.tile_pool(name="wpool", bufs=4))
            xpool = ctx.enter_context(tc.tile_pool(name="xpool", bufs=2))
            qtp = ctx.enter_context(tc.tile_pool(name="qtp", bufs=1))
            ktp = ctx.enter_context(tc.tile_pool(name="ktp", bufs=2))
            vgp = ctx.enter_context(tc.tile_pool(name="vgp", bufs=2))
            accp = ctx.enter_context(tc.tile_pool(name="accp", bufs=1))
            ctp = ctx.enter_context(tc.tile_pool(name="ctp", bufs=1))
            attnp = ctx.enter_context(tc.tile_pool(name="attnp", bufs=6))
            bcp = ctx.enter_context(tc.tile_pool(name="bcp", bufs=2))
            outp = ctx.enter_context(tc.tile_pool(name="outp", bufs=2))
            drp = ctx.enter_context(tc.tile_pool(name="drp", bufs=2, space="DRAM"))
            sps = ctx.enter_context(tc.tile_pool(name="sps", bufs=2, space="PSUM"))
            cps = ctx.enter_context(tc.tile_pool(name="cps", bufs=2, space="PSUM"))

            # ---- constants ----
            bq_t = const.tile([P, KC], F32, tag="bq")
            nc.sync.dma_start(bq_t[:], bq)
            bk_t = const.tile([P, KC], F32, tag="bk")
            nc.sync.dma_start(bk_t[:], bk)
            bo_t = const.tile([P, KC], F32, tag="bo")
            nc.sync.dma_start(bo_t[:], bo)
            bv_bc = const.tile([P, D], F32, tag="bvb")
            nc.sync.dma_start(bv_bc[:], bvb)

            QT = qtp.tile([P, KC, R], BF16, tag="qt")
            CT = ctp.tile([P, KC, R], BF16, tag="ct")
            # 64 ctx rows + softmax-sum row per head (all partition-0 based:
            # multi-input DVE ops require inputs to share a start partition)
            ctxacc = accp.tile([65, H, 2, 512], F32, tag="acc")

            # ---------- Q^T projection (rows = xT tiles 0,1) ----------
            # psum [128,1024] = two m2 banks; start clears has_written for
            # the whole bank, so one start per m2-bank; k-inner accumulates.
            for rt in range(2):
                xf = xpool.tile([P, KC, 512], BF16, tag="x")
                nc.sync.dma_start(xf[:, 0:4], xTt[:, rt, 0:4])
                nc.sync.dma_start(xf[:, 4:8], xTt[:, rt, 4:8])
                for mp in range(MP):
                    wq = wpool.tile([P, KC, 256], BF16, tag="w")
                    wdma(wq, Wq4, mp)
                    ps = sps.tile([P, 1024], F32, tag="sp")
                    for m2 in range(2):
                        for k in range(KC):
                            nc.tensor.matmul(
                                ps[:, ds(m2 * 512, 512)],
                                wq[:, k, ts(m2, P)], xf[:, k],
                                start=(k == 0), stop=(k == KC - 1),
                                skip_group_check=True)
                    for m2 in range(2):
                        m = 2 * mp + m2
                        nc.vector.tensor_scalar_add(
                            QT[:, m, ts(rt, 512)], ps[:, ts(m2, 512)],
                            bq_t[:, m:m + 1])

            # half-proj biases: loaded after the Q-proj DMAs so the
            # startup descriptor order stays identical to the tuned v2
            bkh_t = const.tile([P, 4], F32, tag="bkh")
            nc.sync.dma_start(bkh_t[:], bkh)
            bvh_t = const.tile([P, 512], F32, tag="bvh")
            nc.sync.dma_start(bvh_t[:], bvbh)

            # ---------- K/V projection units (yield per matmul) ----------
            def gen_k_unit(kt_tile, xb, wk, mp, bias):
                # K^T for one weight m-pair over this block's 512 keys
                ps = sps.tile([P, 1024], F32, tag="sp")
                for m2 in range(2):
                    for k in range(KC):
                        nc.tensor.matmul(
                            ps[:, ds(m2 * 512, 512)],
                            wk[:, k, ts(m2, P)], xb[:, k],
                            start=(k == 0), stop=(k == KC - 1),
                            skip_group_check=True)
                        yield
                for m2 in range(2):
                    m = 2 * mp + m2
                    nc.vector.tensor_scalar_add(
                        kt_tile[:, m, :], ps[:, ts(m2, 512)], bias[:, m:m + 1])

            def gen_v_unit(vaug, xb, wv2, ktp2, vbias=None):
                # V (natural) for two key tiles x one 512-wide v-col pair
                ntp = wv2[2]
                if vbias is None:
                    vbias = bv_bc[:, ds(ntp * 512, 512)]
                ps = sps.tile([P, 1024], F32, tag="sp")
                for kh in range(2):
                    kt = 2 * ktp2 + kh
                    for k in range(KC):
                        for hh in range(2):
                            nc.tensor.matmul(
                                ps[:, ds(kh * 512 + hh * 256, 256)],
                                xb[:, k, ts(kt, P)],
                                wv2[hh][:, k],
                                start=(k == 0 and hh == 0),
                                stop=(k == KC - 1 and hh == 1),
                                skip_group_check=True)
                            yield
                h0 = ntp * 8
                for kh in range(2):
                    kt = 2 * ktp2 + kh
                    vdst = vaug[:, kt, :].rearrange(
                        "p (h c) -> p h c", c=65)[:, h0:h0 + 8, 0:64]
                    nc.vector.tensor_tensor(
                        vdst,
                        ps[:, ts(kh, 512)].rearrange("p (h c) -> p h c", c=HD),
                        vbias.rearrange("p (h c) -> p h c", c=HD),
                        mybir.AluOpType.add)

            def write_ones(vaug):
                ones_view = vaug[:].rearrange(
                    "p kt (h c) -> p kt h c", c=65)[:, :, :, 64:65]
                nc.vector.tensor_scalar(
                    ones_view,
                    bv_bc[:, 0:KTB * H].rearrange(
                        "p (kt h) -> p kt h", kt=KTB).unsqueeze(3),
                    0.0, 1.0, mybir.AluOpType.mult, mybir.AluOpType.add)

            # ---- K/V pair exchange (blocks 1-3): each core computes the
            # half of K^T/V for its parity (host supplies Wk4h/Wv4h as that
            # half in global head order), stages it in kt_tile[:, 0:4] /
            # vaug heads 0-7, round-trips through an HBM AllGather over the
            # core pair, and imports BOTH halves in global order (its own
            # half rewrites identical bytes). write_ones runs post-import.
            CCB = 4 * 512 + 4 * 520   # K half + V half, bf16 elems per part

            def gen_proj_half(b1, i):
                if i < 2:
                    wk = wpool.tile([P, KC, 256], BF16, tag="w",
                                    name=f"wkh{b1}_{i}")
                    nc.sync.dma_start(wk[:], Wk4h[:, i])
                    yield from gen_k_unit(kv[b1][0], xb_next, wk, i, bkh_t)
                else:
                    ktp2 = i - 2
                    if ktp2 == 0:
                        wv2 = []
                        for hh in range(2):
                            wv = wpool.tile([P, KC, 256], BF16, tag="w",
                                            name=f"wvh{b1}_{hh}")
                            nc.sync.dma_start(wv[:], Wv4h[:, hh])
                            wv2.append(wv)
                        wv2.append(0)
                        wv_state[0] = wv2
                    yield from gen_v_unit(kv[b1][1], xb_next, wv_state[0],
                                          ktp2, vbias=bvh_t[:])

            def emit_exchange(b1):
                kt_tile, vaug = kv[b1]
                ccin = drp.tile([P, CCB], BF16, tag="ci", name=f"ci{b1}")
                ccout = drp.tile([2, P, CCB], BF16, tag="co", name=f"co{b1}")
                nc.sync.dma_start(ccin[:, 0:2048], kt_tile[:, 0:4, :])
                nc.sync.dma_start(
                    ccin[:, 2048:CCB].rearrange("p (kt c) -> p kt c", kt=KTB),
                    vaug[:, :, 0:520])
                nc.gpsimd.collective_compute(
                    "AllGather",
                    mybir.AluOpType.bypass,
                    replica_groups=[[0, 1], [2, 3], [4, 5], [6, 7]],
                    ins=[ccin[:]],
                    outs=[ccout[:]],
                )
                for r in range(2):
                    nc.sync.dma_start(kt_tile[:, 4 * r:4 * r + 4, :],
                                      ccout[r, :, 0:2048])
                    nc.sync.dma_start(
                        vaug[:, :, 520 * r:520 * (r + 1)],
                        ccout[r, :, 2048:CCB].rearrange(
                            "p (kt c) -> p kt c", kt=KTB))
                write_ones(vaug)

            def make_kv_tiles(b):
                kt_tile = ktp.tile([P, KC, SBK], BF16, tag="kt", name=f"KT{b}")
                vaug = vgp.tile([P, KTB, H * 65], BF16, tag="vg", name=f"VG{b}")
                return kt_tile, vaug

            def load_xb(b):
                xf = xpool.tile([P, KC, 512], BF16, tag="x", name=f"xb{b}")
                nc.sync.dma_start(xf[:, 0:4], xTtK[:, b, 0:4])
                nc.sync.dma_start(xf[:, 4:8], xTtK[:, b, 4:8])
                return xf

            def gen_proj_unit(b1, i):
                # blocks 1-3: half projection in slots 0-3, exchange at 4
                if i < 4:
                    yield from gen_proj_half(b1, i)
                elif i == 4:
                    emit_exchange(b1)
                    return
                    yield

            # ---------- block 0 K/V projection (no attention to overlap) ----
            kv = [None] * NB
            kv[0] = make_kv_tiles(0)
            xb0 = load_xb(0)
            for mp in range(MP):
                wk = wpool.tile([P, KC, 256], BF16, tag="w", name=f"wk0_{mp}")
                wdma(wk, Wk4, mp)
                for _ in gen_k_unit(kv[0][0], xb0, wk, mp, bk_t):
                    pass
            for ntp in range(2):
                wv2 = []
                for hh in range(2):
                    wv = wpool.tile([P, KC, 256], BF16, tag="w",
                                    name=f"wv0_{ntp}{hh}")
                    wdma(wv, Wv4, 2 * ntp + hh)
                    wv2.append(wv)
                wv2.append(ntp)
                for ktp2 in range(2):
                    for _ in gen_v_unit(kv[0][1], xb0, wv2, ktp2):
                        pass
            write_ones(kv[0][1])

            # ---------- attention: hp slots, both rt, proj(b+1) woven in ----
            wo_tiles = {}

            def gen_scores(bb, hp, supers):
                # per (head, kt-pair): two supers (rt0, rt1); each stationary
                # (head, kt) serves both rt matmuls back-to-back
                kt_tile = kv[bb][0]
                for head in range(2):
                    po = 64 * head
                    for p2 in range(2):
                        sup = [sps.tile([P, 1024], F32, tag="sp",
                                        name=f"sup{head}{p2}r{rt}")
                               for rt in range(2)]
                        for kh in range(2):
                            kt = 2 * p2 + kh
                            for rt in range(2):
                                nc.tensor.matmul(
                                    sup[rt][:, ts(kh, 512)],
                                    kt_tile[po:po + 64, hp, ts(kt, P)],
                                    QT[po:po + 64, hp, ts(rt, 512)],
                                    start=True, stop=True,
                                    tile_position=(po, 0))
                                yield
                        for rt in range(2):
                            at = attnp.tile([P, 1024], BF16, tag="a")
                            nc.scalar.activation(at[:], sup[rt][:], FP.Exp,
                                                 scale=0.125)
                            supers.append(at)

            def gen_ctx(bb, hp, supers):
                vaug = kv[bb][1]
                for head in range(2):
                    h = 2 * hp + head
                    po = head * 64
                    cp = cps.tile([65, 1024], F32, tag="cp")
                    for kt in range(KTB):
                        for rt in range(2):
                            at = supers[head * 4 + (kt // 2) * 2 + rt]
                            nc.tensor.matmul(
                                cp[:, ts(rt, 512)],
                                vaug[:, kt, ds(h * 65, 65)],
                                at[:, ts(kt % 2, 512)],
                                start=(kt == 0), stop=(kt == KTB - 1))
                            yield
                    accv = ctxacc[:, h].rearrange("p a b -> p (a b)")
                    if bb == 0:
                        nc.vector.tensor_copy(accv, cp[:])
                    else:
                        nc.vector.tensor_add(accv, accv, cp[:])
                    if bb == NB - 1:
                        # normalize into CT: stage the sums row (contiguity +
                        # partition-0 base), broadcast it across 64 partitions
                        # on GpSimd, reciprocal at 64-partition parallelism,
                        # one multiply for both rt halves.
                        bcs = bcp.tile([1, 1024], F32, tag="sa")
                        nc.vector.tensor_copy(
                            bcs[:], ctxacc[64:65, h].rearrange(
                                "p a b -> p (a b)"))
                        bcb = bcp.tile([64, 1024], F32, tag="sb")
                        nc.gpsimd.partition_broadcast(bcb[:], bcs[:])
                        rec = bcp.tile([64, 1024], F32, tag="sb2")
                        nc.vector.reciprocal_approx_fast(rec[:], bcb[:])
                        nc.vector.tensor_mul(
                            CT[po:po + 64, hp],
                            ctxacc[0:64, h].rearrange("p a b -> p (a b)"),
                            rec[:])

            def interleave(gens):
                # round-robin; each entry is (generator, instrs-per-turn).
                # chunking keeps same-mode matmuls back-to-back (a PE mode
                # switch fp32r/bf16/row_grp costs ~100ns of pipeline drain).
                gens = [[g, c] for g, c in gens if g is not None]
                while gens:
                    alive = []
                    for gc in gens:
                        g, c = gc
                        ok = True
                        for _ in range(c):
                            try:
                                next(g)
                            except StopIteration:
                                ok = False
                                break
                        if ok:
                            alive.append(gc)
                    gens = alive

            pending = []   # (bb, hp, supers) awaiting ctx; LAG 1 slot
            wv_state = [None]
            for b in range(NB):
                if b + 1 < NB:
                    kv[b + 1] = make_kv_tiles(b + 1)
                    xb_next = load_xb(b + 1)
                for si, hp in enumerate(range(HP)):
                    supers = []
                    pending.append((b, hp, supers))
                    gsc = gen_scores(b, hp, supers)
                    gpr = gen_proj_unit(b + 1, hp) if b + 1 < NB else None
                    gcx = None
                    if len(pending) > 1:
                        gcx = gen_ctx(*pending.pop(0))
                    if b == NB - 1 and 1 <= si < 5:
                        # prefetch Wo during the last block
                        mp = si - 1
                        wo = wpool.tile([P, KC, 256], BF16, tag="w",
                                        name=f"wo_{mp}")
                        wdma(wo, Wo4, mp)
                        wo_tiles[mp] = wo
                    interleave([(gsc, 4), (gpr, 8), (gcx, 8)])

            # ---------- out^T = (ctx @ Wo)^T + bo ----------
            def gen_outproj():
                for mp in range(MP):
                    wo = wo_tiles[mp]
                    for rt in range(2):
                        ps = sps.tile([P, 1024], F32, tag="sp")
                        for m2 in range(2):
                            for k in range(KC):
                                nc.tensor.matmul(
                                    ps[:, ts(m2, 512)], wo[:, k, ts(m2, P)],
                                    CT[:, k, ts(rt, 512)],
                                    start=(k == 0), stop=(k == KC - 1))
                                yield
                        for m2 in range(2):
                            m = 2 * mp + m2
                            ob = outp.tile([P, 512], BF16, tag="ob")
                            nc.vector.tensor_scalar_add(
                                ob[:], ps[:, ts(m2, 512)], bo_t[:, m:m + 1])
                            nc.sync.dma_start(outT[ts(m, P), ts(rt, 512)],
                                              ob[:])

            # the final ctx drain interleaves with out-proj: out-proj chunks
            # k=0..6 cover the last head's normalize latency before any
            # out-proj matmul needs CT chunk 7.
            drains = [gen_ctx(*ent) for ent in pending]
            interleave([(g, 10 ** 6) for g in drains] + [(gen_outproj(), 16)])

    nc.compile()
    _CACHED["nc"] = nc
    return nc


def make_in_maps(x, Wq, bq, Wk, bk, Wv, bv, Wo, bo):
    x = np.asarray(x, dtype=np.float32)
    B = x.shape[0]

    def bcol(b):
        return np.ascontiguousarray(np.asarray(b, np.float32).reshape(KC, P).T)

    def w4(w, dt=ml_dtypes.bfloat16):
        w = np.asarray(w, np.float32).reshape(KC, P, MP, 256)
        return np.ascontiguousarray(w.transpose(1, 2, 0, 3).astype(dt))

    wq4, wk4, wv4, wo4 = w4(Wq), w4(Wk), w4(Wv), w4(Wo)
    wk4h = [np.ascontiguousarray(wk4[:, 0:2]), np.ascontiguousarray(wk4[:, 2:4])]
    wv4h = [np.ascontiguousarray(wv4[:, 0:2]), np.ascontiguousarray(wv4[:, 2:4])]
    bq2, bk2, bo2 = bcol(bq), bcol(bk), bcol(bo)
    bkh2 = [np.ascontiguousarray(bk2[:, 0:4]), np.ascontiguousarray(bk2[:, 4:8])]
    bv1 = np.ascontiguousarray(np.asarray(bv, np.float32).reshape(1, D))

    bvb = np.ascontiguousarray(np.tile(bv1, (P, 1)))
    in_maps = []
    for c in range(8):
        b, half = c // 2, c % 2
        xb = x[b]
        if half == 1:
            xb = np.concatenate([xb[R:], xb[:R]], axis=0)

        def tiled(a):
            return np.ascontiguousarray(
                a.reshape(4, 512, KC, P).transpose(3, 0, 2, 1)
                .astype(ml_dtypes.bfloat16))

        # queries: rotated so own rows sit in tiles 0-1; keys: global order
        # (the K/V pair exchange requires both cores to cover the same keys)
        xtt = tiled(xb)
        xttk = tiled(x[b])
        in_maps.append({
            "xTt": xtt, "xTtK": xttk,
            "Wq4": wq4, "Wk4": wk4, "Wv4": wv4, "Wo4": wo4,
            "Wk4h": wk4h[half], "Wv4h": wv4h[half],
            "bkh": bkh2[half],
            "bvbh": np.ascontiguousarray(bvb[:, half * 512:(half + 1) * 512]),
            "bq": bq2, "bk": bk2, "bo": bo2, "bvb": bvb,
        })
    return in_maps


def assemble_out(results, B):
    out = np.empty((B, S, D), dtype=np.float32)
    for c in range(8):
        b, half = c // 2, c % 2
        out[b, half * R:(half + 1) * R, :] = results[c]["outT"].T.astype(np.float32)
    return out


def kernel(x, Wq, bq, Wk, bk, Wv, bv, Wo, bo, **kw):
    nc = build()
    in_maps = make_in_maps(x, Wq, bq, Wk, bk, Wv, bv, Wo, bo)
    res = run_bass_kernel_spmd(nc, in_maps, core_ids=list(range(8)))
    return assemble_out(res.results, np.asarray(x).shape[0])


# revision 21
# speedup vs baseline: 1.0021x; 1.0021x over previous
"""Multi-head attention (B=4,S=2048,D=1024,H=16,Hd=64, fp32) on 8 TRN2 NeuronCores.

Sharding: core c handles batch b=c//2 and query-row half h=c%2 (1024 rows).
Each core computes Q for its 1024 rows, attends all 16 heads over the full
2048 keys, and does the output projection. K/V for block 0 is computed fully
on both cores of a pair; for key blocks 1-3 each core computes only its
parity's half of the d-model columns (host supplies Wk4h/Wv4h as that half
in global head order) and the halves are exchanged through an HBM AllGather
over core pairs {2b,2b+1}, issued four slots before the block needs them so
the ~40us collective latency hides under the previous block's attention.
Queries use a host-rotated x stream (own rows in tiles 0-1); the K/V path
uses an unrotated x stream so both pair members cover identical keys.

Schedule notes (trace-driven): x and all weights stream as bf16 (same
1 cycle/row PE rate as f32r at 512-wide moving, half the DMA + SBUF).
Back-to-back same-mode matmuls run at theory speed but tile-boundary
semaphore waits and fp32r/bf16/row_grp mode switches cost ~100ns, so the
slot interleave is chunked (scores 4 / proj 8 / ctx 8 instructions per
turn). Softmax normalization is batched per head: stage the sums row,
GpSimd partition-broadcast [64,1024], reciprocal at 64-partition
parallelism, one multiply, pipelined via bufs=2 pools. The startup DMA
descriptor order (consts, then x, then weights, split halves) is
empirically load-bearing: reordering it flips the part into a ~1.2x slower
clock regime for the whole run.
"""
import numpy as np
import ml_dtypes
from contextlib import ExitStack

import concourse.bass as bass
import concourse.tile as tile
from concourse import bacc, mybir
from concourse.bass import ts, ds
from concourse.bass_utils import run_bass_kernel_spmd

P = 128
D = 1024
KC = 8                 # contraction chunks of 128
S = 2048               # keys per batch
R = 1024               # query rows per core
NB = 4                 # key blocks
SBK = S // NB          # 512 keys per block
KTB = SBK // P         # 4 key tiles per block
H = 16
HP = H // 2            # 8 head pairs
HD = 64
MP = 4                 # weight DMA tiles of 256 output-cols
BF16 = mybir.dt.bfloat16
F32 = mybir.dt.float32
FP = mybir.ActivationFunctionType

_CACHED = {}


def build():
    if "nc" in _CACHED:
        return _CACHED["nc"]
    nc = bacc.Bacc("TRN2", target_bir_lowering=False, debug=False, num_devices=8)
    xTt = nc.dram_tensor("xTt", [P, 4, KC, 512], BF16, kind="ExternalInput").ap()
    xTtK = nc.dram_tensor("xTtK", [P, 4, KC, 512], BF16, kind="ExternalInput").ap()
    Wq4 = nc.dram_tensor("Wq4", [P, MP, KC, 256], BF16, kind="ExternalInput").ap()
    Wk4 = nc.dram_tensor("Wk4", [P, MP, KC, 256], BF16, kind="ExternalInput").ap()
    Wv4 = nc.dram_tensor("Wv4", [P, MP, KC, 256], BF16, kind="ExternalInput").ap()
    Wo4 = nc.dram_tensor("Wo4", [P, MP, KC, 256], BF16, kind="ExternalInput").ap()
    Wk4h = nc.dram_tensor("Wk4h", [P, 2, KC, 256], BF16, kind="ExternalInput").ap()
    Wv4h = nc.dram_tensor("Wv4h", [P, 2, KC, 256], BF16, kind="ExternalInput").ap()
    bkh = nc.dram_tensor("bkh", [P, 4], F32, kind="ExternalInput").ap()
    bvbh = nc.dram_tensor("bvbh", [P, 512], F32, kind="ExternalInput").ap()
    bq = nc.dram_tensor("bq", [P, KC], F32, kind="ExternalInput").ap()
    bk = nc.dram_tensor("bk", [P, KC], F32, kind="ExternalInput").ap()
    bo = nc.dram_tensor("bo", [P, KC], F32, kind="ExternalInput").ap()
    bvb = nc.dram_tensor("bvb", [P, D], F32, kind="ExternalInput").ap()
    outT = nc.dram_tensor("outT", [D, R], BF16, kind="ExternalOutput").ap()

    def wdma(wt, src, mp):
        nc.sync.dma_start(wt[:, 0:4], src[:, mp, 0:4])
        nc.sync.dma_start(wt[:, 4:8], src[:, mp, 4:8])

    with tile.TileContext(nc) as tc:
        with ExitStack() as ctx:
            const = ctx.enter_context(tc.tile_pool(name="const", bufs=1))
            wpool = ctx.enter_context(tc.tile_pool(name="wpool", bufs=4))
            xpool = ctx.enter_context(tc.tile_pool(name="xpool", bufs=2))
            qtp = ctx.enter_context(tc.tile_pool(name="qtp", bufs=1))
            ktp = ctx.enter_context(tc.tile_pool(name="ktp", bufs=2))
            vgp = ctx.enter_context(tc.tile_pool(name="vgp", bufs=2))
            accp = ctx.enter_context(tc.tile_pool(name="accp", bufs=1))
            ctp = ctx.enter_context(tc.tile_pool(name="ctp", bufs=1))
            attnp = ctx.enter_context(tc.tile_pool(name="attnp", bufs=6))
            bcp = ctx.enter_context(tc.tile_pool(name="bcp", bufs=2))
            outp = ctx.enter_context(tc.tile_pool(name="outp", bufs=2))
            drp = ctx.enter_context(tc.tile_pool(name="drp", bufs=2, space="DRAM"))
            sps = ctx.enter_context(tc.tile_pool(name="sps", bufs=2, space="PSUM"))
            cps = ctx.enter_context(tc.tile_pool(name="cps", bufs=2, space="PSUM"))

            # ---- constants ----
            bq_t = const.tile([P, KC], F32, tag="bq")
            nc.sync.dma_start(bq_t[:], bq)
            bk_t = const.tile([P, KC], F32, tag="bk")
            nc.sync.dma_start(bk_t[:], bk)
            bo_t = const.tile([P, KC], F32, tag="bo")
            nc.sync.dma_start(bo_t[:], bo)
            bv_bc = const.tile([P, D], F32, tag="bvb")
            nc.sync.dma_start(bv_bc[:], bvb)

            QT = qtp.tile([P, KC, R], BF16, tag="qt")
            CT = ctp.tile([P, KC, R], BF16, tag="ct")
            # 64 ctx rows + softmax-sum row per head (all partition-0 based:
            # multi-input DVE ops require inputs to share a start partition)
            ctxacc = accp.tile([65, H, 2, 512], F32, tag="acc")

            # ---------- Q^T projection (rows = xT tiles 0,1) ----------
            # psum [128,1024] = two m2 banks; start clears has_written for
            # the whole bank, so one start per m2-bank; k-inner accumulates.
            for rt in range(2):
                xf = xpool.tile([P, KC, 512], BF16, tag="x")
                nc.sync.dma_start(xf[:, 0:4], xTt[:, rt, 0:4])
                nc.sync.dma_start(xf[:, 4:8], xTt[:, rt, 4:8])
                for mp in range(MP):
                    wq = wpool.tile([P, KC, 256], BF16, tag="w")
                    wdma(wq, Wq4, mp)
                    ps = sps.tile([P, 1024], F32, tag="sp")
                    for m2 in range(2):
                        for k in range(KC):
                            nc.tensor.matmul(
                                ps[:, ds(m2 * 512, 512)],
                                wq[:, k, ts(m2, P)], xf[:, k],
                                start=(k == 0), stop=(k == KC - 1),
                                skip_group_check=True)
                    for m2 in range(2):
                        m = 2 * mp + m2
                        nc.vector.tensor_scalar_add(
                            QT[:, m, ts(rt, 512)], ps[:, ts(m2, 512)],
                            bq_t[:, m:m + 1])

            # half-proj biases: loaded after the Q-proj DMAs so the
            # startup descriptor order stays identical to the tuned v2
            bkh_t = const.tile([P, 4], F32, tag="bkh")
            nc.sync.dma_start(bkh_t[:], bkh)
            bvh_t = const.tile([P, 512], F32, tag="bvh")
            nc.sync.dma_start(bvh_t[:], bvbh)

            # ---------- K/V projection units (yield per matmul) ----------
            def gen_k_unit(kt_tile, xb, wk, mp, bias):
                # K^T for one weight m-pair over this block's 512 keys
                ps = sps.tile([P, 1024], F32, tag="sp")
                for m2 in range(2):
                    for k in range(KC):
                        nc.tensor.matmul(
                            ps[:, ds(m2 * 512, 512)],
                            wk[:, k, ts(m2, P)], xb[:, k],
                            start=(k == 0), stop=(k == KC - 1),
                            skip_group_check=True)
                        yield
                for m2 in range(2):
                    m = 2 * mp + m2
                    nc.vector.tensor_scalar_add(
                        kt_tile[:, m, :], ps[:, ts(m2, 512)], bias[:, m:m + 1])

            def gen_v_unit(vaug, xb, wv2, ktp2, vbias=None):
                # V (natural) for two key tiles x one 512-wide v-col pair
                ntp = wv2[2]
                if vbias is None:
                    vbias = bv_bc[:, ds(ntp * 512, 512)]
                ps = sps.tile([P, 1024], F32, tag="sp")
                for kh in range(2):
                    kt = 2 * ktp2 + kh
                    for k in range(KC):
                        for hh in range(2):
                            nc.tensor.matmul(
                                ps[:, ds(kh * 512 + hh * 256, 256)],
                                xb[:, k, ts(kt, P)],
                                wv2[hh][:, k],
                                start=(k == 0 and hh == 0),
                                stop=(k == KC - 1 and hh == 1),
                                skip_group_check=True)
                            yield
                h0 = ntp * 8
                for kh in range(2):
                    kt = 2 * ktp2 + kh
                    vdst = vaug[:, kt, :].rearrange(
                        "p (h c) -> p h c", c=65)[:, h0:h0 + 8, 0:64]
                    nc.vector.tensor_tensor(
                        vdst,
                        ps[:, ts(kh, 512)].rearrange("p (h c) -> p h c", c=HD),
                        vbias.rearrange("p (h c) -> p h c", c=HD),
                        mybir.AluOpType.add)

            def write_ones(vaug):
                ones_view = vaug[:].rearrange(
                    "p kt (h c) -> p kt h c", c=65)[:, :, :, 64:65]
                nc.vector.tensor_scalar(
                    ones_view,
                    bv_bc[:, 0:KTB * H].rearrange(
                        "p (kt h) -> p kt h", kt=KTB).unsqueeze(3),
                    0.0, 1.0, mybir.AluOpType.mult, mybir.AluOpType.add)

            # ---- K/V pair exchange (blocks 1-3): each core computes the
            # half of K^T/V for its parity (host supplies Wk4h/Wv4h as that
            # half in global head order), stages it in kt_tile[:, 0:4] /
            # vaug heads 0-7, round-trips through an HBM AllGather over the
            # core pair, and imports BOTH halves in global order (its own
            # half rewrites identical bytes). write_ones runs post-import.
            CCB = 4 * 512 + 4 * 520   # K half + V half, bf16 elems per part

            def gen_proj_half(b1, i):
                if i < 2:
                    wk = wpool.tile([P, KC, 256], BF16, tag="w",
                                    name=f"wkh{b1}_{i}")
                    nc.sync.dma_start(wk[:], Wk4h[:, i])
                    yield from gen_k_unit(kv[b1][0], xb_next, wk, i, bkh_t)
                else:
                    ktp2 = i - 2
                    if ktp2 == 0:
                        wv2 = []
                        for hh in range(2):
                            wv = wpool.tile([P, KC, 256], BF16, tag="w",
                                            name=f"wvh{b1}_{hh}")
                            nc.sync.dma_start(wv[:], Wv4h[:, hh])
                            wv2.append(wv)
                        wv2.append(0)
                        wv_state[0] = wv2
                    yield from gen_v_unit(kv[b1][1], xb_next, wv_state[0],
                                          ktp2, vbias=bvh_t[:])

            def emit_exchange(b1):
                kt_tile, vaug = kv[b1]
                ccin = drp.tile([P, CCB], BF16, tag="ci", name=f"ci{b1}")
                ccout = drp.tile([2, P, CCB], BF16, tag="co", name=f"co{b1}")
                nc.sync.dma_start(ccin[:, 0:2048], kt_tile[:, 0:4, :])
                nc.sync.dma_start(
                    ccin[:, 2048:CCB].rearrange("p (kt c) -> p kt c", kt=KTB),
                    vaug[:, :, 0:520])
                nc.gpsimd.collective_compute(
                    "AllGather",
                    mybir.AluOpType.bypass,
                    replica_groups=[[0, 1], [2, 3], [4, 5], [6, 7]],
                    ins=[ccin[:]],
                    outs=[ccout[:]],
                )
                for r in range(2):
                    nc.sync.dma_start(kt_tile[:, 4 * r:4 * r + 4, :],
                                      ccout[r, :, 0:2048])
                    nc.sync.dma_start(
                        vaug[:, :, 520 * r:520 * (r + 1)],
                        ccout[r, :, 2048:CCB].rearrange(
                            "p (kt c) -> p kt c", kt=KTB))
                write_ones(vaug)

            def make_kv_tiles(b):
                kt_tile = ktp.tile([P, KC, SBK], BF16, tag="kt", name=f"KT{b}")
                vaug = vgp.tile([P, KTB, H * 65], BF16, tag="vg", name=f"VG{b}")
                return kt_tile, vaug

            def load_xb(b):
                xf = xpool.tile([P, KC, 512], BF16, tag="x", name=f"xb{b}")
                nc.sync.dma_start(xf[:, 0:4], xTtK[:, b, 0:4])
                nc.sync.dma_start(xf[:, 4:8], xTtK[:, b, 4:8])
                return xf

            def gen_proj_unit(b1, i):
                # blocks 1-3: half projection in slots 0-3, exchange at 4
                if i < 4:
                    yield from gen_proj_half(b1, i)
                elif i == 4:
                    emit_exchange(b1)
                    return
                    yield

            # ---------- block 0 K/V projection (no attention to overlap) ----
            kv = [None] * NB
            kv[0] = make_kv_tiles(0)
            xb0 = load_xb(0)
            for mp in range(MP):
                wk = wpool.tile([P, KC, 256], BF16, tag="w", name=f"wk0_{mp}")
                wdma(wk, Wk4, mp)
                for _ in gen_k_unit(kv[0][0], xb0, wk, mp, bk_t):
                    pass
            for ntp in range(2):
                wv2 = []
                for hh in range(2):
                    wv = wpool.tile([P, KC, 256], BF16, tag="w",
                                    name=f"wv0_{ntp}{hh}")
                    wdma(wv, Wv4, 2 * ntp + hh)
                    wv2.append(wv)
                wv2.append(ntp)
                for ktp2 in range(2):
                    for _ in gen_v_unit(kv[0][1], xb0, wv2, ktp2):
                        pass
            write_ones(kv[0][1])

            # ---------- attention: hp slots, both rt, proj(b+1) woven in ----
            wo_tiles = {}

            def gen_scores(bb, hp, supers):
                # per (head, kt-pair): two supers (rt0, rt1); each stationary
                # (head, kt) serves both rt matmuls back-to-back
                kt_tile = kv[bb][0]
                for head in range(2):
                    po = 64 * head
                    for p2 in range(2):
                        sup = [sps.tile([P, 1024], F32, tag="sp",
                                        name=f"sup{head}{p2}r{rt}")
                               for rt in range(2)]
                        for kh in range(2):
                            kt = 2 * p2 + kh
                            for rt in range(2):
                                nc.tensor.matmul(
                                    sup[rt][:, ts(kh, 512)],
                                    kt_tile[po:po + 64, hp, ts(kt, P)],
                                    QT[po:po + 64, hp, ts(rt, 512)],
                                    start=True, stop=True,
                                    tile_position=(po, 0))
                                yield
                        for rt in range(2):
                            at = attnp.tile([P, 1024], BF16, tag="a")
                            nc.scalar.activation(at[:], sup[rt][:], FP.Exp,
                                                 scale=0.125)
                            supers.append(at)

            def gen_ctx(bb, hp, supers):
                vaug = kv[bb][1]
                for head in range(2):
                    h = 2 * hp + head
                    po = head * 64
                    cp = cps.tile([65, 1024], F32, tag="cp")
                    for kt in range(KTB):
                        for rt in range(2):
                            at = supers[head * 4 + (kt // 2) * 2 + rt]
                            nc.tensor.matmul(
                                cp[:, ts(rt, 512)],
                                vaug[:, kt, ds(h * 65, 65)],
                                at[:, ts(kt % 2, 512)],
                                start=(kt == 0), stop=(kt == KTB - 1))
                            yield
                    accv = ctxacc[:, h].rearrange("p a b -> p (a b)")
                    if bb == 0:
                        nc.vector.tensor_copy(accv, cp[:])
                    else:
                        nc.vector.tensor_add(accv, accv, cp[:])
                    if bb == NB - 1:
                        # normalize into CT: stage the sums row (contiguity +
                        # partition-0 base), broadcast it across 64 partitions
                        # on GpSimd, reciprocal at 64-partition parallelism,
                        # one multiply for both rt halves.
                        bcs = bcp.tile([1, 1024], F32, tag="sa")
                        nc.vector.tensor_copy(
                            bcs[:], ctxacc[64:65, h].rearrange(
                                "p a b -> p (a b)"))
                        bcb = bcp.tile([64, 1024], F32, tag="sb")
                        nc.gpsimd.partition_broadcast(bcb[:], bcs[:])
                        rec = bcp.tile([64, 1024], F32, tag="sb2")
                        nc.vector.reciprocal_approx_fast(rec[:], bcb[:])
                        nc.vector.tensor_mul(
                            CT[po:po + 64, hp],
                            ctxacc[0:64, h].rearrange("p a b -> p (a b)"),
                            rec[:])

            def interleave(gens):
                # round-robin; each entry is (generator, instrs-per-turn).
                # chunking keeps same-mode matmuls back-to-back (a PE mode
                # switch fp32r/bf16/row_grp costs ~100ns of pipeline drain).
                gens = [[g, c] for g, c in gens if g is not None]
                while gens:
                    alive = []
                    for gc in gens:
                        g, c = gc
                        ok = True
                        for _ in range(c):
                            try:
                                next(g)
                            except StopIteration:
                                ok = False
                                break
                        if ok:
                            alive.append(gc)
                    gens = alive

            pending = []   # (bb, hp, supers) awaiting ctx; LAG 1 slot
            wv_state = [None]
            for b in range(NB):
                if b + 1 < NB:
                    kv[b + 1] = make_kv_tiles(b + 1)
                    xb_next = load_xb(b + 1)
                for si, hp in enumerate(range(HP)):
                    supers = []
                    pending.append((b, hp, supers))
                    gsc = gen_scores(b, hp, supers)
                    gpr = gen_proj_unit(b + 1, hp) if b + 1 < NB else None
                    gcx = None
                    if len(pending) > 1:
                        gcx = gen_ctx(*pending.pop(0))
                    if b == NB - 1 and 1 <= si < 5:
                        # prefetch Wo during the last block
                        mp = si - 1
                        wo = wpool.tile([P, KC, 256], BF16, tag="w",
                                        name=f"wo_{mp}")
                        wdma(wo, Wo4, mp)
                        wo_tiles[mp] = wo
                    interleave([(gsc, 4), (gpr, 8), (gcx, 8)])

            # ---------- out^T = (ctx @ Wo)^T + bo ----------
            def gen_outproj():
                for mp in range(MP):
                    wo = wo_tiles[mp]
                    for rt in range(2):
                        ps = sps.tile([P, 1024], F32, tag="sp")
                        for m2 in range(2):
                            for k in range(KC):
                                nc.tensor.matmul(
                                    ps[:, ts(m2, 512)], wo[:, k, ts(m2, P)],
                                    CT[:, k, ts(rt, 512)],
                                    start=(k == 0), stop=(k == KC - 1))
                                yield
                        for m2 in range(2):
                            m = 2 * mp + m2
                            ob = outp.tile([P, 512], BF16, tag="ob")
                            nc.vector.tensor_scalar_add(
                                ob[:], ps[:, ts(m2, 512)], bo_t[:, m:m + 1])
                            nc.sync.dma_start(outT[ts(m, P), ts(rt, 512)],
                                              ob[:])

            # the final ctx drain interleaves with out-proj: out-proj chunks
            # k=0..6 cover the last head's normalize latency before any
            # out-proj matmul needs CT chunk 7.
            drains = [gen_ctx(*ent) for ent in pending]
            interleave([(g, 10 ** 6) for g in drains] + [(gen_outproj(), 16)])

    nc.compile()
    _CACHED["nc"] = nc
    return nc


def make_in_maps(x, Wq, bq, Wk, bk, Wv, bv, Wo, bo):
    x = np.asarray(x, dtype=np.float32)
    B = x.shape[0]

    def bcol(b):
        return np.ascontiguousarray(np.asarray(b, np.float32).reshape(KC, P).T)

    def w4(w, dt=ml_dtypes.bfloat16):
        w = np.asarray(w, np.float32).reshape(KC, P, MP, 256)
        return np.ascontiguousarray(w.transpose(1, 2, 0, 3).astype(dt))

    wq4, wk4, wv4, wo4 = w4(Wq), w4(Wk), w4(Wv), w4(Wo)
    wk4h = [np.ascontiguousarray(wk4[:, 0:2]), np.ascontiguousarray(wk4[:, 2:4])]
    wv4h = [np.ascontiguousarray(wv4[:, 0:2]), np.ascontiguousarray(wv4[:, 2:4])]
    bq2, bk2, bo2 = bcol(bq), bcol(bk), bcol(bo)
    bkh2 = [np.ascontiguousarray(bk2[:, 0:4]), np.ascontiguousarray(bk2[:, 4:8])]
    bv1 = np.ascontiguousarray(np.asarray(bv, np.float32).reshape(1, D))

    bvb = np.ascontiguousarray(np.tile(bv1, (P, 1)))
    in_maps = []
    for c in range(8):
        b, half = c // 2, c % 2
        xb = x[b]
        if half == 1:
            xb = np.concatenate([xb[R:], xb[:R]], axis=0)

        def tiled(a):
            return np.ascontiguousarray(
                a.reshape(4, 512, KC, P).transpose(3, 0, 2, 1)
                .astype(ml_dtypes.bfloat16))

        # queries: rotated so own rows sit in tiles 0-1; keys: global order
        # (the K/V pair exchange requires both cores to cover the same keys)
        xtt = tiled(xb)
        xttk = tiled(x[b])
        in_maps.append({
            "xTt": xtt, "xTtK": xttk,
            "Wq4": wq4, "Wk4": wk4, "Wv4": wv4, "Wo4": wo4,
            "Wk4h": wk4h[half], "Wv4h": wv4h[half],
            "bkh": bkh2[half],
            "bvbh": np.ascontiguousarray(bvb[:, half * 512:(half + 1) * 512]),
            "bq": bq2, "bk": bk2, "bo": bo2, "bvb": bvb,
        })
    return in_maps


def assemble_out(results, B):
    out = np.empty((B, S, D), dtype=np.float32)
    for c in range(8):
        b, half = c // 2, c % 2
        out[b, half * R:(half + 1) * R, :] = results[c]["outT"].T.astype(np.float32)
    return out


def kernel(x, Wq, bq, Wk, bk, Wv, bv, Wo, bo, **kw):
    nc = build()
    in_maps = make_in_maps(x, Wq, bq, Wk, bk, Wv, bv, Wo, bo)
    res = run_bass_kernel_spmd(nc, in_maps, core_ids=list(range(8)))
    return assemble_out(res.results, np.asarray(x).shape[0])


# revision 24
# speedup vs baseline: 1.0411x; 1.0389x over previous
"""Multi-head attention (B=4,S=2048,D=1024,H=16,Hd=64, fp32) on 8 TRN2 NeuronCores.

Sharding: core c handles batch b=c//2 and query-row half h=c%2 (1024 rows).
Each core computes Q for its 1024 rows, attends all 16 heads over the full
2048 keys, and does the output projection. K/V for block 0 is computed fully
on both cores of a pair; for key blocks 1-3 each core computes only its
parity's half of the d-model columns (host supplies Wk4h/Wv4h as that half
in global head order) and the halves are exchanged through an HBM AllGather
over core pairs {2b,2b+1}, issued four slots before the block needs them so
the ~40us collective latency hides under the previous block's attention.
Queries use a host-rotated x stream (own rows in tiles 0-1); the K/V path
uses an unrotated x stream so both pair members cover identical keys.

Schedule notes (trace-driven): x and all weights stream as bf16 (same
1 cycle/row PE rate as f32r at 512-wide moving, half the DMA + SBUF).
Back-to-back same-mode matmuls run at theory speed but tile-boundary
semaphore waits and fp32r/bf16/row_grp mode switches cost ~100ns, so the
slot interleave is chunked (scores 4 / proj 8 / ctx 8 instructions per
turn). Softmax normalization is batched per head: stage the sums row (the
copy to partition 0 is mandatory - partition_broadcast reads absolute
partition 0; DVE divide is rejected by the backend), GpSimd-broadcast
[64,1024], reciprocal at 64-partition parallelism, one multiply. Block 3
runs slot hp=7 first and out-proj contracts in order [7,0..6] so the last
normalization chain overlaps out-proj instead of stalling it. The startup
DMA descriptor order (consts, then x, then weights, split halves) is
empirically load-bearing: reordering it flips the part into a ~1.2x slower
clock regime for the whole run.
"""
import numpy as np
import ml_dtypes
from contextlib import ExitStack

import concourse.bass as bass
import concourse.tile as tile
from concourse import bacc, mybir
from concourse.bass import ts, ds
from concourse.bass_utils import run_bass_kernel_spmd

P = 128
D = 1024
KC = 8                 # contraction chunks of 128
S = 2048               # keys per batch
R = 1024               # query rows per core
NB = 4                 # key blocks
SBK = S // NB          # 512 keys per block
KTB = SBK // P         # 4 key tiles per block
H = 16
HP = H // 2            # 8 head pairs
HD = 64
MP = 4                 # weight DMA tiles of 256 output-cols
BF16 = mybir.dt.bfloat16
F32 = mybir.dt.float32
FP = mybir.ActivationFunctionType

_CACHED = {}


def build():
    if "nc" in _CACHED:
        return _CACHED["nc"]
    nc = bacc.Bacc("TRN2", target_bir_lowering=False, debug=False, num_devices=8)
    xTt = nc.dram_tensor("xTt", [P, 4, KC, 512], BF16, kind="ExternalInput").ap()
    xTtK = nc.dram_tensor("xTtK", [P, 4, KC, 512], BF16, kind="ExternalInput").ap()
    Wq4 = nc.dram_tensor("Wq4", [P, MP, KC, 256], BF16, kind="ExternalInput").ap()
    Wk4 = nc.dram_tensor("Wk4", [P, MP, KC, 256], BF16, kind="ExternalInput").ap()
    Wv4 = nc.dram_tensor("Wv4", [P, MP, KC, 256], BF16, kind="ExternalInput").ap()
    Wo4 = nc.dram_tensor("Wo4", [P, MP, KC, 256], BF16, kind="ExternalInput").ap()
    Wk4h = nc.dram_tensor("Wk4h", [P, 2, KC, 256], BF16, kind="ExternalInput").ap()
    Wv4h = nc.dram_tensor("Wv4h", [P, 2, KC, 256], BF16, kind="ExternalInput").ap()
    bkh = nc.dram_tensor("bkh", [P, 4], F32, kind="ExternalInput").ap()
    bvbh = nc.dram_tensor("bvbh", [P, 512], F32, kind="ExternalInput").ap()
    bq = nc.dram_tensor("bq", [P, KC], F32, kind="ExternalInput").ap()
    bk = nc.dram_tensor("bk", [P, KC], F32, kind="ExternalInput").ap()
    bo = nc.dram_tensor("bo", [P, KC], F32, kind="ExternalInput").ap()
    bvb = nc.dram_tensor("bvb", [P, D], F32, kind="ExternalInput").ap()
    outT = nc.dram_tensor("outT", [D, R], BF16, kind="ExternalOutput").ap()

    def wdma(wt, src, mp):
        nc.sync.dma_start(wt[:, 0:4], src[:, mp, 0:4])
        nc.sync.dma_start(wt[:, 4:8], src[:, mp, 4:8])

    with tile.TileContext(nc) as tc:
        with ExitStack() as ctx:
            const = ctx.enter_context(tc.tile_pool(name="const", bufs=1))
            wpool = ctx.enter_context(tc.tile_pool(name="wpool", bufs=4))
            xpool = ctx.enter_context(tc.tile_pool(name="xpool", bufs=2))
            qtp = ctx.enter_context(tc.tile_pool(name="qtp", bufs=1))
            ktp = ctx.enter_context(tc.tile_pool(name="ktp", bufs=2))
            vgp = ctx.enter_context(tc.tile_pool(name="vgp", bufs=2))
            accp = ctx.enter_context(tc.tile_pool(name="accp", bufs=1))
            ctp = ctx.enter_context(tc.tile_pool(name="ctp", bufs=1))
            attnp = ctx.enter_context(tc.tile_pool(name="attnp", bufs=6))
            bcp = ctx.enter_context(tc.tile_pool(name="bcp", bufs=2))
            outp = ctx.enter_context(tc.tile_pool(name="outp", bufs=2))
            drp = ctx.enter_context(tc.tile_pool(name="drp", bufs=2, space="DRAM"))
            sps = ctx.enter_context(tc.tile_pool(name="sps", bufs=2, space="PSUM"))
            cps = ctx.enter_context(tc.tile_pool(name="cps", bufs=2, space="PSUM"))

            # ---- constants ----
            bq_t = const.tile([P, KC], F32, tag="bq")
            nc.sync.dma_start(bq_t[:], bq)
            bk_t = const.tile([P, KC], F32, tag="bk")
            nc.sync.dma_start(bk_t[:], bk)
            bo_t = const.tile([P, KC], F32, tag="bo")
            nc.sync.dma_start(bo_t[:], bo)
            bv_bc = const.tile([P, D], F32, tag="bvb")
            nc.sync.dma_start(bv_bc[:], bvb)

            QT = qtp.tile([P, KC, R], BF16, tag="qt")
            CT = ctp.tile([P, KC, R], BF16, tag="ct")
            # 64 ctx rows + softmax-sum row per head (all partition-0 based:
            # multi-input DVE ops require inputs to share a start partition)
            ctxacc = accp.tile([65, H, 2, 512], F32, tag="acc")

            # ---------- Q^T projection (rows = xT tiles 0,1) ----------
            # psum [128,1024] = two m2 banks; start clears has_written for
            # the whole bank, so one start per m2-bank; k-inner accumulates.
            for rt in range(2):
                xf = xpool.tile([P, KC, 512], BF16, tag="x")
                nc.sync.dma_start(xf[:, 0:4], xTt[:, rt, 0:4])
                nc.sync.dma_start(xf[:, 4:8], xTt[:, rt, 4:8])
                for mp in range(MP):
                    wq = wpool.tile([P, KC, 256], BF16, tag="w")
                    wdma(wq, Wq4, mp)
                    ps = sps.tile([P, 1024], F32, tag="sp")
                    for m2 in range(2):
                        for k in range(KC):
                            nc.tensor.matmul(
                                ps[:, ds(m2 * 512, 512)],
                                wq[:, k, ts(m2, P)], xf[:, k],
                                start=(k == 0), stop=(k == KC - 1),
                                skip_group_check=True)
                    for m2 in range(2):
                        m = 2 * mp + m2
                        nc.vector.tensor_scalar_add(
                            QT[:, m, ts(rt, 512)], ps[:, ts(m2, 512)],
                            bq_t[:, m:m + 1])

            # half-proj biases: loaded after the Q-proj DMAs so the
            # startup descriptor order stays identical to the tuned v2
            bkh_t = const.tile([P, 4], F32, tag="bkh")
            nc.sync.dma_start(bkh_t[:], bkh)
            bvh_t = const.tile([P, 512], F32, tag="bvh")
            nc.sync.dma_start(bvh_t[:], bvbh)

            # ---------- K/V projection units (yield per matmul) ----------
            def gen_k_unit(kt_tile, xb, wk, mp, bias):
                # K^T for one weight m-pair over this block's 512 keys
                ps = sps.tile([P, 1024], F32, tag="sp")
                for m2 in range(2):
                    for k in range(KC):
                        nc.tensor.matmul(
                            ps[:, ds(m2 * 512, 512)],
                            wk[:, k, ts(m2, P)], xb[:, k],
                            start=(k == 0), stop=(k == KC - 1),
                            skip_group_check=True)
                        yield
                for m2 in range(2):
                    m = 2 * mp + m2
                    nc.vector.tensor_scalar_add(
                        kt_tile[:, m, :], ps[:, ts(m2, 512)], bias[:, m:m + 1])

            def gen_v_unit(vaug, xb, wv2, ktp2, vbias=None):
                # V (natural) for two key tiles x one 512-wide v-col pair
                ntp = wv2[2]
                if vbias is None:
                    vbias = bv_bc[:, ds(ntp * 512, 512)]
                ps = sps.tile([P, 1024], F32, tag="sp")
                for kh in range(2):
                    kt = 2 * ktp2 + kh
                    for k in range(KC):
                        for hh in range(2):
                            nc.tensor.matmul(
                                ps[:, ds(kh * 512 + hh * 256, 256)],
                                xb[:, k, ts(kt, P)],
                                wv2[hh][:, k],
                                start=(k == 0 and hh == 0),
                                stop=(k == KC - 1 and hh == 1),
                                skip_group_check=True)
                            yield
                h0 = ntp * 8
                for kh in range(2):
                    kt = 2 * ktp2 + kh
                    vdst = vaug[:, kt, :].rearrange(
                        "p (h c) -> p h c", c=65)[:, h0:h0 + 8, 0:64]
                    nc.vector.tensor_tensor(
                        vdst,
                        ps[:, ts(kh, 512)].rearrange("p (h c) -> p h c", c=HD),
                        vbias.rearrange("p (h c) -> p h c", c=HD),
                        mybir.AluOpType.add)

            def write_ones(vaug):
                ones_view = vaug[:].rearrange(
                    "p kt (h c) -> p kt h c", c=65)[:, :, :, 64:65]
                nc.vector.tensor_scalar(
                    ones_view,
                    bv_bc[:, 0:KTB * H].rearrange(
                        "p (kt h) -> p kt h", kt=KTB).unsqueeze(3),
                    0.0, 1.0, mybir.AluOpType.mult, mybir.AluOpType.add)

            # ---- K/V pair exchange (blocks 1-3): each core computes the
            # half of K^T/V for its parity (host supplies Wk4h/Wv4h as that
            # half in global head order), stages it in kt_tile[:, 0:4] /
            # vaug heads 0-7, round-trips through an HBM AllGather over the
            # core pair, and imports BOTH halves in global order (its own
            # half rewrites identical bytes). write_ones runs post-import.
            CCB = 4 * 512 + 4 * 520   # K half + V half, bf16 elems per part

            def gen_proj_half(b1, i):
                if i < 2:
                    wk = wpool.tile([P, KC, 256], BF16, tag="w",
                                    name=f"wkh{b1}_{i}")
                    nc.sync.dma_start(wk[:], Wk4h[:, i])
                    yield from gen_k_unit(kv[b1][0], xb_next, wk, i, bkh_t)
                else:
                    ktp2 = i - 2
                    if ktp2 == 0:
                        wv2 = []
                        for hh in range(2):
                            wv = wpool.tile([P, KC, 256], BF16, tag="w",
                                            name=f"wvh{b1}_{hh}")
                            nc.sync.dma_start(wv[:], Wv4h[:, hh])
                            wv2.append(wv)
                        wv2.append(0)
                        wv_state[0] = wv2
                    yield from gen_v_unit(kv[b1][1], xb_next, wv_state[0],
                                          ktp2, vbias=bvh_t[:])

            def emit_exchange(b1):
                kt_tile, vaug = kv[b1]
                ccin = drp.tile([P, CCB], BF16, tag="ci", name=f"ci{b1}")
                ccout = drp.tile([2, P, CCB], BF16, tag="co", name=f"co{b1}")
                nc.sync.dma_start(ccin[:, 0:2048], kt_tile[:, 0:4, :])
                nc.sync.dma_start(
                    ccin[:, 2048:CCB].rearrange("p (kt c) -> p kt c", kt=KTB),
                    vaug[:, :, 0:520])
                nc.gpsimd.collective_compute(
                    "AllGather",
                    mybir.AluOpType.bypass,
                    replica_groups=[[0, 1], [2, 3], [4, 5], [6, 7]],
                    ins=[ccin[:]],
                    outs=[ccout[:]],
                )
                for r in range(2):
                    nc.sync.dma_start(kt_tile[:, 4 * r:4 * r + 4, :],
                                      ccout[r, :, 0:2048])
                    nc.sync.dma_start(
                        vaug[:, :, 520 * r:520 * (r + 1)],
                        ccout[r, :, 2048:CCB].rearrange(
                            "p (kt c) -> p kt c", kt=KTB))
                write_ones(vaug)

            def make_kv_tiles(b):
                kt_tile = ktp.tile([P, KC, SBK], BF16, tag="kt", name=f"KT{b}")
                vaug = vgp.tile([P, KTB, H * 65], BF16, tag="vg", name=f"VG{b}")
                return kt_tile, vaug

            def load_xb(b):
                xf = xpool.tile([P, KC, 512], BF16, tag="x", name=f"xb{b}")
                nc.sync.dma_start(xf[:, 0:4], xTtK[:, b, 0:4])
                nc.sync.dma_start(xf[:, 4:8], xTtK[:, b, 4:8])
                return xf

            def gen_proj_unit(b1, i):
                # blocks 1-3: half projection in slots 0-3, exchange at 4
                if i < 4:
                    yield from gen_proj_half(b1, i)
                elif i == 4:
                    emit_exchange(b1)
                    return
                    yield

            # ---------- block 0 K/V projection (no attention to overlap) ----
            kv = [None] * NB
            kv[0] = make_kv_tiles(0)
            xb0 = load_xb(0)
            for mp in range(MP):
                wk = wpool.tile([P, KC, 256], BF16, tag="w", name=f"wk0_{mp}")
                wdma(wk, Wk4, mp)
                for _ in gen_k_unit(kv[0][0], xb0, wk, mp, bk_t):
                    pass
            for ntp in range(2):
                wv2 = []
                for hh in range(2):
                    wv = wpool.tile([P, KC, 256], BF16, tag="w",
                                    name=f"wv0_{ntp}{hh}")
                    wdma(wv, Wv4, 2 * ntp + hh)
                    wv2.append(wv)
                wv2.append(ntp)
                for ktp2 in range(2):
                    for _ in gen_v_unit(kv[0][1], xb0, wv2, ktp2):
                        pass
            write_ones(kv[0][1])

            # ---------- attention: hp slots, both rt, proj(b+1) woven in ----
            wo_tiles = {}

            def gen_scores(bb, hp, supers):
                # per (head, kt-pair): two supers (rt0, rt1); each stationary
                # (head, kt) serves both rt matmuls back-to-back
                kt_tile = kv[bb][0]
                for head in range(2):
                    po = 64 * head
                    for p2 in range(2):
                        sup = [sps.tile([P, 1024], F32, tag="sp",
                                        name=f"sup{head}{p2}r{rt}")
                               for rt in range(2)]
                        for kh in range(2):
                            kt = 2 * p2 + kh
                            for rt in range(2):
                                nc.tensor.matmul(
                                    sup[rt][:, ts(kh, 512)],
                                    kt_tile[po:po + 64, hp, ts(kt, P)],
                                    QT[po:po + 64, hp, ts(rt, 512)],
                                    start=True, stop=True,
                                    tile_position=(po, 0))
                                yield
                        for rt in range(2):
                            at = attnp.tile([P, 1024], BF16, tag="a")
                            nc.scalar.activation(at[:], sup[rt][:], FP.Exp,
                                                 scale=0.125)
                            supers.append(at)

            def gen_ctx(bb, hp, supers):
                vaug = kv[bb][1]
                for head in range(2):
                    h = 2 * hp + head
                    po = head * 64
                    cp = cps.tile([65, 1024], F32, tag="cp")
                    for kt in range(KTB):
                        for rt in range(2):
                            at = supers[head * 4 + (kt // 2) * 2 + rt]
                            nc.tensor.matmul(
                                cp[:, ts(rt, 512)],
                                vaug[:, kt, ds(h * 65, 65)],
                                at[:, ts(kt % 2, 512)],
                                start=(kt == 0), stop=(kt == KTB - 1))
                            yield
                    accv = ctxacc[:, h].rearrange("p a b -> p (a b)")
                    if bb == 0:
                        nc.vector.tensor_copy(accv, cp[:])
                    else:
                        nc.vector.tensor_add(accv, accv, cp[:])
                    if bb == NB - 1:
                        # normalize into CT: stage the sums row (contiguity +
                        # partition-0 base), broadcast it across 64 partitions
                        # on GpSimd, reciprocal at 64-partition parallelism,
                        # one multiply for both rt halves.
                        bcs = bcp.tile([1, 1024], F32, tag="sa")
                        nc.vector.tensor_copy(
                            bcs[:], ctxacc[64:65, h].rearrange(
                                "p a b -> p (a b)"))
                        bcb = bcp.tile([64, 1024], F32, tag="sb")
                        nc.gpsimd.partition_broadcast(bcb[:], bcs[:])
                        rec = bcp.tile([64, 1024], F32, tag="sb2")
                        nc.vector.reciprocal_approx_fast(rec[:], bcb[:])
                        nc.vector.tensor_mul(
                            CT[po:po + 64, hp],
                            ctxacc[0:64, h].rearrange("p a b -> p (a b)"),
                            rec[:])

            def interleave(gens):
                # round-robin; each entry is (generator, instrs-per-turn).
                # chunking keeps same-mode matmuls back-to-back (a PE mode
                # switch fp32r/bf16/row_grp costs ~100ns of pipeline drain).
                gens = [[g, c] for g, c in gens if g is not None]
                while gens:
                    alive = []
                    for gc in gens:
                        g, c = gc
                        ok = True
                        for _ in range(c):
                            try:
                                next(g)
                            except StopIteration:
                                ok = False
                                break
                        if ok:
                            alive.append(gc)
                    gens = alive

            pending = []   # (bb, hp, supers) awaiting ctx; LAG 1 slot
            wv_state = [None]
            for b in range(NB):
                if b + 1 < NB:
                    kv[b + 1] = make_kv_tiles(b + 1)
                    xb_next = load_xb(b + 1)
                # block 3 runs hp=7 first so CT chunk 7 normalizes early;
                # chunk 6 (drained last) is the out-proj groups' final k.
                hps = [7, 0, 1, 2, 3, 4, 5, 6] if b == NB - 1 else range(HP)
                for si, hp in enumerate(hps):
                    supers = []
                    pending.append((b, hp, supers))
                    gsc = gen_scores(b, hp, supers)
                    gpr = gen_proj_unit(b + 1, hp) if b + 1 < NB else None
                    gcx = None
                    if len(pending) > 1:
                        gcx = gen_ctx(*pending.pop(0))
                    if b == NB - 1 and 1 <= si < 5:
                        # prefetch Wo during the last block
                        mp = si - 1
                        wo = wpool.tile([P, KC, 256], BF16, tag="w",
                                        name=f"wo_{mp}")
                        wdma(wo, Wo4, mp)
                        wo_tiles[mp] = wo
                    interleave([(gsc, 4), (gpr, 8), (gcx, 8)])

            # ---------- out^T = (ctx @ Wo)^T + bo ----------
            # contraction order [7,0..6] matches CT-chunk readiness under the
            # block-3 slot rotation (chunk 6 lands last, from the drain).
            KSEQ = [7, 0, 1, 2, 3, 4, 5, 6]

            def gen_outproj():
                for mp in range(MP):
                    wo = wo_tiles[mp]
                    for rt in range(2):
                        ps = sps.tile([P, 1024], F32, tag="sp")
                        for m2 in range(2):
                            for ki, k in enumerate(KSEQ):
                                nc.tensor.matmul(
                                    ps[:, ts(m2, 512)], wo[:, k, ts(m2, P)],
                                    CT[:, k, ts(rt, 512)],
                                    start=(ki == 0), stop=(ki == KC - 1))
                                yield
                        for m2 in range(2):
                            m = 2 * mp + m2
                            ob = outp.tile([P, 512], BF16, tag="ob")
                            nc.vector.tensor_scalar_add(
                                ob[:], ps[:, ts(m2, 512)], bo_t[:, m:m + 1])
                            nc.sync.dma_start(outT[ts(m, P), ts(rt, 512)],
                                              ob[:])

            # the final ctx drain interleaves with out-proj: out-proj chunks
            # k=0..6 cover the last head's normalize latency before any
            # out-proj matmul needs CT chunk 7.
            drains = [gen_ctx(*ent) for ent in pending]
            interleave([(g, 10 ** 6) for g in drains] + [(gen_outproj(), 16)])

    nc.compile()
    _CACHED["nc"] = nc
    return nc


def make_in_maps(x, Wq, bq, Wk, bk, Wv, bv, Wo, bo):
    x = np.asarray(x, dtype=np.float32)
    B = x.shape[0]

    def bcol(b):
        return np.ascontiguousarray(np.asarray(b, np.float32).reshape(KC, P).T)

    def w4(w, dt=ml_dtypes.bfloat16):
        w = np.asarray(w, np.float32).reshape(KC, P, MP, 256)
        return np.ascontiguousarray(w.transpose(1, 2, 0, 3).astype(dt))

    wq4, wk4, wv4, wo4 = w4(Wq), w4(Wk), w4(Wv), w4(Wo)
    wk4h = [np.ascontiguousarray(wk4[:, 0:2]), np.ascontiguousarray(wk4[:, 2:4])]
    wv4h = [np.ascontiguousarray(wv4[:, 0:2]), np.ascontiguousarray(wv4[:, 2:4])]
    bq2, bk2, bo2 = bcol(bq), bcol(bk), bcol(bo)
    bkh2 = [np.ascontiguousarray(bk2[:, 0:4]), np.ascontiguousarray(bk2[:, 4:8])]
    bv1 = np.ascontiguousarray(np.asarray(bv, np.float32).reshape(1, D))

    bvb = np.ascontiguousarray(np.tile(bv1, (P, 1)))
    in_maps = []
    for c in range(8):
        b, half = c // 2, c % 2
        xb = x[b]
        if half == 1:
            xb = np.concatenate([xb[R:], xb[:R]], axis=0)

        def tiled(a):
            return np.ascontiguousarray(
                a.reshape(4, 512, KC, P).transpose(3, 0, 2, 1)
                .astype(ml_dtypes.bfloat16))

        # queries: rotated so own rows sit in tiles 0-1; keys: global order
        # (the K/V pair exchange requires both cores to cover the same keys)
        xtt = tiled(xb)
        xttk = tiled(x[b])
        in_maps.append({
            "xTt": xtt, "xTtK": xttk,
            "Wq4": wq4, "Wk4": wk4, "Wv4": wv4, "Wo4": wo4,
            "Wk4h": wk4h[half], "Wv4h": wv4h[half],
            "bkh": bkh2[half],
            "bvbh": np.ascontiguousarray(bvb[:, half * 512:(half + 1) * 512]),
            "bq": bq2, "bk": bk2, "bo": bo2, "bvb": bvb,
        })
    return in_maps


def assemble_out(results, B):
    out = np.empty((B, S, D), dtype=np.float32)
    for c in range(8):
        b, half = c // 2, c % 2
        out[b, half * R:(half + 1) * R, :] = results[c]["outT"].T.astype(np.float32)
    return out


def kernel(x, Wq, bq, Wk, bk, Wv, bv, Wo, bo, **kw):
    nc = build()
    in_maps = make_in_maps(x, Wq, bq, Wk, bk, Wv, bv, Wo, bo)
    res = run_bass_kernel_spmd(nc, in_maps, core_ids=list(range(8)))
    return assemble_out(res.results, np.asarray(x).shape[0])
